# revision 3
# baseline (speedup 1.0000x reference)
"""Gaussian-splat blend kernel v4 — scan + culling + paired transposes.

Like v3 (per-tile top-GA gaussian culling, scan-based transmittance) with:
  - GA=63, blocks of 64 (63 real + 1 reset pad): two px-tiles' T-blocks
    fit one [128,128] PE transpose -> half the transposes, half the
    PSUM->SBUF copy volume.
  - Even px-tiles contract st rows 0:63, odd rows 64:127; their mm3
    outputs go to SEPARATE PSUM banks (PE row-strip mixing within one
    bank faults the device).
  - exp merged over 2 supersteps ([128, 1008] PSUM reads).
"""

import numpy as np
import ml_dtypes

import concourse.bass as bass
import concourse.bacc as bacc
import concourse.mybir as mybir
import concourse.tile as tile
from concourse.bass_utils import run_bass_kernel_spmd

G = 128
GA = 63                     # gaussians kept per tile
B = 4
N = 65536
BN = B * N
NCORES = 8
PPC = BN // NCORES          # 32768
NT = PPC // 128             # 256 tiles/core
NTG = BN // 128             # 2048 tiles global
SS = 8                      # px tiles per superstep
NSUP = NT // SS             # 32
SUPPX = SS * 128            # 1024
BLK = GA + 1                # 64
GRP = 4
NGRP = NSUP // GRP          # 8
WSS = SS * BLK              # 512
WGRP = GRP * WSS            # 2048
CSPLIT = 320                # stp copy cols on ACT (rest on DVE)

F32 = mybir.dt.float32
BF16 = mybir.dt.bfloat16
F16 = mybir.dt.float16
AFT = mybir.ActivationFunctionType
ALU = mybir.AluOpType
BF = ml_dtypes.bfloat16
NPF16 = np.float16

PROFILE = False
LAST_EXEC_NS = None
LAST_RESULTS = None

_cached = None


def _build():
    nc = bacc.Bacc("TRN2", target_bir_lowering=False, debug=False,
                   num_devices=NCORES)
    f18 = nc.dram_tensor("f18", [18, PPC], BF16, kind="ExternalInput")
    c18a = nc.dram_tensor("c18a", [18, NT * GA], BF16, kind="ExternalInput")
    d16a = nc.dram_tensor("d16a", [128, NT * 3], F16, kind="ExternalInput")
    zconst = nc.dram_tensor("zconst", [128, WGRP], F16, kind="ExternalInput")
    ident = nc.dram_tensor("ident", [128, 128], F16, kind="ExternalInput")
    # per group: [128, 48] even-parity block then [128, 48] odd
    out = nc.dram_tensor("out", [128, NGRP * 96], F32, kind="ExternalOutput")

    with tile.TileContext(nc) as tc:
        with (
            tc.tile_pool(name="const", bufs=1) as constp,
            tc.tile_pool(name="featp", bufs=4) as featp,
            tc.tile_pool(name="cbp", bufs=4) as cbp,
            tc.tile_pool(name="zp", bufs=3, space="PSUM") as zp,
            tc.tile_pool(name="stpp", bufs=2, space="PSUM") as stpp,
            tc.tile_pool(name="moutp", bufs=1, space="PSUM") as moutp,
            tc.tile_pool(name="btp", bufs=2) as btp,
            tc.tile_pool(name="tp", bufs=2) as tp_,
            tc.tile_pool(name="stp", bufs=3) as stp_,
            tc.tile_pool(name="obp", bufs=2) as obp,
        ):
            dummy = constp.tile([1, 8], F32)
            nc.gpsimd.memset(dummy[:], 0.0)
            nc.scalar.activation(dummy[:], dummy[:], AFT.Exp)

            fbufs = [featp.tile([18, SUPPX], BF16, tag="fbuf",
                                name=f"fbuf{i}") for i in range(NSUP)]
            cbufs = [cbp.tile([18, SS * GA], BF16, tag="cbuf",
                              name=f"cbuf{i}") for i in range(NSUP)]
            nc.sync.dma_start(fbufs[0][:], f18[:, bass.ts(0, SUPPX)])
            nc.sync.dma_start(cbufs[0][:], c18a[:, bass.ts(0, SS * GA)])
            d16_t = constp.tile([128, NT * 3], F16)
            nc.sync.dma_start(d16_t[:], d16a[:])
            zc_t = constp.tile([128, WGRP], F16)
            nc.gpsimd.dma_start(zc_t[:], zconst[:])
            id_t = constp.tile([128, 128], F16)
            nc.gpsimd.dma_start(id_t[:], ident[:])

            a4s = [constp.tile([128, GRP * SS, BLK], F16, name=f"a4_{i}")
                   for i in range(2)]
            for a4 in a4s:
                nc.gpsimd.memset(a4[:, :, GA:BLK], 1.0)

            tts = {}
            state = {}

            def emit_front(s):
                fbuf, cbuf = fbufs[s], cbufs[s]
                if s > 0:
                    nc.gpsimd.dma_start(fbuf[:], f18[:, bass.ts(s, SUPPX)])
                    nc.sync.dma_start(cbuf[:], c18a[:, bass.ts(s, SS * GA)])
                z2 = zp.tile([128, SS * GA], F32, name="z2")
                for i in range(SS):
                    nc.tensor.matmul(
                        z2[:, bass.ts(i, GA)],
                        fbuf[:, bass.ts(i, 128)],
                        cbuf[:, bass.ts(i, GA)], start=True, stop=True)
                a4 = a4s[(s // GRP) % 2]
                b0 = (s % GRP) * SS
                nc.scalar.activation(
                    a4[:, b0:b0 + SS, 0:GA],
                    z2.rearrange("p (b c) -> p b c", c=GA)[:], AFT.Exp)
                if s % GRP == GRP - 1:
                    a4 = a4s[(s // GRP) % 2]
                    a4f = a4.rearrange("p b c -> p (b c)")
                    bt = btp.tile([128, WGRP], F16)
                    nc.vector.tensor_scalar_sub(bt[:], a4f[:], 1.0)
                    tt = tp_.tile([128, WGRP], F16)
                    nc.vector.tensor_tensor_scan(
                        tt[:], bt[:], zc_t[:],
                        initial=-1.0, op0=ALU.mult, op1=ALU.add)
                    tts[s // GRP] = tt

            def emit_back(s):
                tt = tts[s // GRP]
                off = (s % GRP) * WSS
                stp = stpp.tile([128, SS // 2 * 128], F16, name="stp")
                for p in range(SS // 2):
                    nc.tensor.transpose(
                        stp[:, bass.ts(p, 128)],
                        tt[:, off + p * 128:off + (p + 1) * 128], id_t[:])
                st = stp_.tile([128, SS // 2 * 128], F16, name="st")
                nc.scalar.activation(st[:, 0:CSPLIT], stp[:, 0:CSPLIT],
                                     AFT.Copy)
                if CSPLIT < SS // 2 * 128:
                    nc.vector.tensor_copy(st[:, CSPLIT:], stp[:, CSPLIT:])
                j = s % GRP
                if j == 0:
                    state['mA'] = moutp.tile([128, GRP * 12], F32,
                                             name="moutA")
                    state['mB'] = moutp.tile([128, GRP * 12], F32,
                                             name="moutB")
                mA, mB = state['mA'], state['mB']
                for p in range(SS // 2):
                    t_even = s * SS + 2 * p
                    t_odd = t_even + 1
                    nc.tensor.matmul(
                        mA[:, j * 12 + p * 3:j * 12 + p * 3 + 3],
                        st[0:GA, bass.ts(p, 128)],
                        d16_t[0:GA, t_even * 3:t_even * 3 + 3],
                        start=True, stop=True)
                    nc.tensor.matmul(
                        mB[:, j * 12 + p * 3:j * 12 + p * 3 + 3],
                        st[64:64 + GA, bass.ts(p, 128)],
                        d16_t[64:64 + GA, t_odd * 3:t_odd * 3 + 3],
                        start=True, stop=True)
                if j == GRP - 1:
                    g = s // GRP
                    obA = obp.tile([128, GRP * 12], F32, tag="ob",
                                   name=f"obA{g}")
                    obB = obp.tile([128, GRP * 12], F32, tag="ob",
                                   name=f"obB{g}")
                    nc.scalar.activation(obA[:], mA[:], AFT.Copy)
                    nc.scalar.activation(obB[:], mB[:], AFT.Copy)
                    nc.sync.dma_start(out[:, g * 96:g * 96 + 48], obA[:])
                    nc.sync.dma_start(out[:, g * 96 + 48:g * 96 + 96], obB[:])

            for s in range(NSUP + GRP):
                if s < NSUP:
                    emit_front(s)
                if s >= GRP:
                    emit_back(s - GRP)

    nc.compile()
    return nc


def _host_prep(x, mu, chol, alpha, rgb, rotation, translation, projection,
               bg):
    d32 = (mu.astype(np.float32) - translation.astype(np.float32)[None, :])
    dist = np.sqrt(np.sum(d32 * d32, axis=-1, dtype=np.float32))
    order = np.argsort(dist, kind="stable")
    mu = mu.astype(np.float64)[order]
    chol = chol.astype(np.float64)[order]
    alpha = alpha.astype(np.float64)[order]
    rgb = rgb.astype(np.float64)[order]
    rotation = rotation.astype(np.float64)
    translation = translation.astype(np.float64)
    projection = projection.astype(np.float64)
    bg = bg.astype(np.float64)

    inv_rot = rotation.T
    inv_trans = -inv_rot @ translation
    Lg = np.tril(chol) + 0.3 * np.eye(3)
    Sigma = np.einsum("gij,gkj->gik", Lg, Lg)
    mu_cam = np.einsum("ij,gj->gi", inv_rot, mu) + inv_trans
    mu2d = np.einsum("ij,gj->gi", projection, mu_cam)
    P_cam = projection @ inv_rot
    S2 = np.einsum("ij,gjk,lk->gil", P_cam, Sigma, P_cam) + 1e-4 * np.eye(2)
    det = S2[:, 0, 0] * S2[:, 1, 1] - S2[:, 0, 1] * S2[:, 1, 0]
    inv = np.empty((G, 2, 2))
    inv[:, 0, 0] = S2[:, 1, 1]
    inv[:, 0, 1] = -S2[:, 0, 1]
    inv[:, 1, 0] = -S2[:, 1, 0]
    inv[:, 1, 1] = S2[:, 0, 0]
    inv /= det[:, None, None]

    sp_ = np.logaddexp(0.0, alpha)
    wg = sp_ / (1.0 + sp_)
    color = rgb / (1.0 + np.abs(rgb))

    A = inv[:, 0, 0]
    Bc = inv[:, 0, 1] + inv[:, 1, 0]
    C = inv[:, 1, 1]
    m0, m1 = mu2d[:, 0], mu2d[:, 1]
    D = -2 * A * m0 - Bc * m1
    E = -Bc * m0 - 2 * C * m1
    F = A * m0 ** 2 + Bc * m0 * m1 + C * m1 ** 2
    coeffs = -0.5 * np.stack([A, Bc, C, D, E, F], axis=1)
    coeffs[:, 5] += np.log(wg)

    coeffsR = coeffs[::-1].copy()
    colorR = color[::-1].copy()
    mu2dR = mu2d[::-1]
    wgR = wg[::-1]
    trcR = (A + C)[::-1]
    dt2R = (A * C - (Bc / 2) ** 2)[::-1]
    lminR = (trcR - np.sqrt(np.maximum(trcR * trcR - 4 * dt2R, 0))) / 2

    xf = x.reshape(BN, 2).astype(np.float64)
    NB_ = 16
    bx = np.clip(((xf[:, 0] + 2) / 4 * NB_).astype(int), 0, NB_ - 1)
    by = np.clip(((xf[:, 1] + 2) / 4 * NB_).astype(int), 0, NB_ - 1)
    pix_order = np.argsort(by * NB_ + bx, kind="stable")
    xs = xf[pix_order]

    tiles = xs.reshape(NTG, 128, 2)
    tmin = tiles.min(axis=1)
    tmax = tiles.max(axis=1)
    cx = np.clip(mu2dR[None, :, 0], tmin[:, None, 0], tmax[:, None, 0])
    cy = np.clip(mu2dR[None, :, 1], tmin[:, None, 1], tmax[:, None, 1])
    dd = (mu2dR[None, :, 0] - cx) ** 2 + (mu2dR[None, :, 1] - cy) ** 2
    abound = wgR[None, :] * np.exp(-0.5 * lminR[None, :] * dd)
    sel = np.argsort(-abound, axis=1, kind="stable")[:, :GA]
    sel.sort(axis=1)

    coefT = np.ascontiguousarray(coeffsR.T).astype(np.float32)
    ch = coefT.astype(BF)
    cl = (coefT - ch.astype(np.float32)).astype(BF)
    c18 = np.concatenate([ch, ch, cl], axis=0).astype(np.float32)
    c18a = c18[:, sel.reshape(-1)].astype(BF)          # [18, NTG*GA]

    csel = colorR[sel]                                 # [NTG, GA, 3]
    dtile = np.empty((NTG, GA, 3))
    dtile[:, :-1] = csel[:, 1:] - csel[:, :-1]
    dtile[:, -1] = bg[None, :] - csel[:, -1]
    dtile[:, 1::2] *= -1.0
    offs = csel[:, 0].astype(np.float32)

    zconst = np.zeros((128, WGRP), NPF16)
    for i in range(GRP * SS):
        zconst[:, i * BLK + GA] = -1.0

    return (c18a, dtile.astype(NPF16), zconst, np.eye(128, dtype=NPF16),
            offs, pix_order, xs)


def kernel(x, mu, chol, alpha, rgb, rotation, translation, projection,
           background_color):
    global _cached, LAST_EXEC_NS, LAST_RESULTS
    x = np.asarray(x, np.float32)
    c18a, dtile, zconst, ident, offs, pix_order, xs = _host_prep(
        x, np.asarray(mu), np.asarray(chol), np.asarray(alpha),
        np.asarray(rgb), np.asarray(rotation), np.asarray(translation),
        np.asarray(projection), np.asarray(background_color))

    feat = np.empty((6, BN), np.float32)
    feat[0] = xs[:, 0] ** 2
    feat[1] = xs[:, 0] * xs[:, 1]
    feat[2] = xs[:, 1] ** 2
    feat[3] = xs[:, 0]
    feat[4] = xs[:, 1]
    feat[5] = 1.0
    fh = feat.astype(BF)
    fl = (feat - fh.astype(np.float32)).astype(BF)
    f18 = np.concatenate([fh, fl, fh], axis=0)

    if _cached is None:
        _cached = _build()
    nc = _cached

    in_maps = []
    for k in range(NCORES):
        d16a = np.zeros((128, NT * 3), NPF16)
        dk = dtile[k * NT:(k + 1) * NT]                # [NT, GA, 3]
        dk2 = dk.transpose(1, 0, 2).reshape(GA, NT * 3)
        d16a[0:GA] = dk2
        d16a[64:64 + GA] = dk2
        in_maps.append({
            "f18": np.ascontiguousarray(f18[:, k * PPC:(k + 1) * PPC]),
            "c18a": np.ascontiguousarray(
                c18a[:, k * NT * GA:(k + 1) * NT * GA]),
            "d16a": d16a,
            "zconst": zconst,
            "ident": ident,
        })

    kwargs = {}
    if PROFILE:
        kwargs = dict(trace=True)
    res = run_bass_kernel_spmd(nc, in_maps, core_ids=list(range(NCORES)),
                               **kwargs)
    LAST_EXEC_NS = res.exec_time_ns
    LAST_RESULTS = res

    parts = []
    for k in range(NCORES):
        # out cols: [group, parity, j, pair, c]; px tile = group*32 +
        # j*8 + pair*2 + parity
        arr = res.results[k]["out"].reshape(128, NGRP, 2, GRP, SS // 2, 3)
        arr = arr.transpose(5, 1, 3, 4, 2, 0)  # [3, g, j, pair, parity, lane]
        parts.append(arr.reshape(3, PPC))
    sortedv = np.concatenate(parts, axis=1)
    sortedv = sortedv + offs.T.repeat(128, axis=1)
    outp = np.empty_like(sortedv)
    outp[:, pix_order] = sortedv
    return outp.T.reshape(B, N, 3).astype(np.float32)


# revision 4
# speedup vs baseline: 1.0482x; 1.0482x over previous
"""Gaussian-splat blend kernel v4 — scan + culling + paired transposes.

Like v3 (per-tile top-GA gaussian culling, scan-based transmittance) with:
  - GA=63, blocks of 64 (63 real + 1 reset pad): two px-tiles' T-blocks
    fit one [128,128] PE transpose -> half the transposes, half the
    PSUM->SBUF copy volume.
  - Even px-tiles contract st rows 0:63, odd rows 64:127; their mm3
    outputs go to SEPARATE PSUM banks (PE row-strip mixing within one
    bank faults the device).
  - exp merged over 2 supersteps ([128, 1008] PSUM reads).
"""

import numpy as np
import ml_dtypes

import concourse.bass as bass
import concourse.bacc as bacc
import concourse.mybir as mybir
import concourse.tile as tile
from concourse.bass_utils import run_bass_kernel_spmd

G = 128
GA = 63                     # gaussians kept per tile
B = 4
N = 65536
BN = B * N
NCORES = 8
PPC = BN // NCORES          # 32768
NT = PPC // 128             # 256 tiles/core
NTG = BN // 128             # 2048 tiles global
SS = 8                      # px tiles per superstep
NSUP = NT // SS             # 32
SUPPX = SS * 128            # 1024
BLK = GA + 1                # 64
GRP = 4
NGRP = NSUP // GRP          # 8
WSS = SS * BLK              # 512
WGRP = GRP * WSS            # 2048
CSPLIT = 512                # stp copy cols on ACT (rest on DVE)

F32 = mybir.dt.float32
BF16 = mybir.dt.bfloat16
F16 = mybir.dt.float16
AFT = mybir.ActivationFunctionType
ALU = mybir.AluOpType
BF = ml_dtypes.bfloat16
NPF16 = np.float16

PROFILE = False
LAST_EXEC_NS = None
LAST_RESULTS = None

_cached = None


def _build():
    nc = bacc.Bacc("TRN2", target_bir_lowering=False, debug=False,
                   num_devices=NCORES)
    f18 = nc.dram_tensor("f18", [18, PPC], BF16, kind="ExternalInput")
    c18a = nc.dram_tensor("c18a", [18, NT * GA], BF16, kind="ExternalInput")
    d16a = nc.dram_tensor("d16a", [128, NT * 3], F16, kind="ExternalInput")
    zconst = nc.dram_tensor("zconst", [128, WGRP], F16, kind="ExternalInput")
    ident = nc.dram_tensor("ident", [128, 128], F16, kind="ExternalInput")
    # per group: [128, 48] even-parity block then [128, 48] odd
    out = nc.dram_tensor("out", [128, NGRP * 96], F32, kind="ExternalOutput")

    with tile.TileContext(nc) as tc:
        with (
            tc.tile_pool(name="const", bufs=1) as constp,
            tc.tile_pool(name="featp", bufs=4) as featp,
            tc.tile_pool(name="cbp", bufs=4) as cbp,
            tc.tile_pool(name="zp", bufs=3, space="PSUM") as zp,
            tc.tile_pool(name="stpp", bufs=3, space="PSUM") as stpp,
            tc.tile_pool(name="moutp", bufs=1, space="PSUM") as moutp,
            tc.tile_pool(name="btp", bufs=2) as btp,
            tc.tile_pool(name="tp", bufs=2) as tp_,
            tc.tile_pool(name="stp", bufs=3) as stp_,
            tc.tile_pool(name="obp", bufs=2) as obp,
        ):
            dummy = constp.tile([1, 8], F32)
            nc.gpsimd.memset(dummy[:], 0.0)
            nc.scalar.activation(dummy[:], dummy[:], AFT.Exp)

            fbufs = [featp.tile([18, SUPPX], BF16, tag="fbuf",
                                name=f"fbuf{i}") for i in range(NSUP)]
            cbufs = [cbp.tile([18, SS * GA], BF16, tag="cbuf",
                              name=f"cbuf{i}") for i in range(NSUP)]
            nc.sync.dma_start(fbufs[0][:], f18[:, bass.ts(0, SUPPX)])
            nc.sync.dma_start(cbufs[0][:], c18a[:, bass.ts(0, SS * GA)])
            d16_t = constp.tile([128, NT * 3], F16)
            nc.sync.dma_start(d16_t[:], d16a[:])
            zc_t = constp.tile([128, WGRP], F16)
            nc.gpsimd.dma_start(zc_t[:], zconst[:])
            id_t = constp.tile([128, 128], F16)
            nc.gpsimd.dma_start(id_t[:], ident[:])

            a4s = [constp.tile([128, GRP * SS, BLK], F16, name=f"a4_{i}")
                   for i in range(2)]
            for a4 in a4s:
                nc.gpsimd.memset(a4[:, :, GA:BLK], 1.0)

            tts = {}
            state = {}

            def emit_front(s):
                fbuf, cbuf = fbufs[s], cbufs[s]
                if s > 0:
                    nc.gpsimd.dma_start(fbuf[:], f18[:, bass.ts(s, SUPPX)])
                    nc.sync.dma_start(cbuf[:], c18a[:, bass.ts(s, SS * GA)])
                z2 = zp.tile([128, SS * GA], F32, name="z2")
                for i in range(SS):
                    nc.tensor.matmul(
                        z2[:, bass.ts(i, GA)],
                        fbuf[:, bass.ts(i, 128)],
                        cbuf[:, bass.ts(i, GA)], start=True, stop=True)
                a4 = a4s[(s // GRP) % 2]
                b0 = (s % GRP) * SS
                nc.scalar.activation(
                    a4[:, b0:b0 + SS, 0:GA],
                    z2.rearrange("p (b c) -> p b c", c=GA)[:], AFT.Exp)
                # per-superstep b-pass: the group scan only waits on the
                # last quarter instead of a full-group subtract
                if s % GRP == 0:
                    state['bt'] = btp.tile([128, WGRP], F16, name="bt")
                bt = state['bt']
                a4f = a4.rearrange("p b c -> p (b c)")
                qw = WGRP // GRP
                q0 = (s % GRP) * qw
                nc.vector.tensor_scalar_sub(
                    bt[:, q0:q0 + qw], a4f[:, q0:q0 + qw], 1.0)
                if s % GRP == GRP - 1:
                    tt = tp_.tile([128, WGRP], F16)
                    nc.vector.tensor_tensor_scan(
                        tt[:], bt[:], zc_t[:],
                        initial=-1.0, op0=ALU.mult, op1=ALU.add)
                    tts[s // GRP] = tt

            def emit_back(s):
                tt = tts[s // GRP]
                off = (s % GRP) * WSS
                stp = stpp.tile([128, SS // 2 * 128], F16, name="stp")
                for p in range(SS // 2):
                    nc.tensor.transpose(
                        stp[:, bass.ts(p, 128)],
                        tt[:, off + p * 128:off + (p + 1) * 128], id_t[:])
                st = stp_.tile([128, SS // 2 * 128], F16, name="st")
                nc.scalar.activation(st[:, 0:CSPLIT], stp[:, 0:CSPLIT],
                                     AFT.Copy)
                if CSPLIT < SS // 2 * 128:
                    nc.vector.tensor_copy(st[:, CSPLIT:], stp[:, CSPLIT:])
                j = s % GRP
                if j == 0:
                    state['mA'] = moutp.tile([128, GRP * 12], F32,
                                             name="moutA")
                    state['mB'] = moutp.tile([128, GRP * 12], F32,
                                             name="moutB")
                mA, mB = state['mA'], state['mB']
                for p in range(SS // 2):
                    t_even = s * SS + 2 * p
                    t_odd = t_even + 1
                    nc.tensor.matmul(
                        mA[:, j * 12 + p * 3:j * 12 + p * 3 + 3],
                        st[0:GA, bass.ts(p, 128)],
                        d16_t[0:GA, t_even * 3:t_even * 3 + 3],
                        start=True, stop=True)
                    nc.tensor.matmul(
                        mB[:, j * 12 + p * 3:j * 12 + p * 3 + 3],
                        st[64:64 + GA, bass.ts(p, 128)],
                        d16_t[64:64 + GA, t_odd * 3:t_odd * 3 + 3],
                        start=True, stop=True)
                if j == GRP - 1:
                    g = s // GRP
                    obA = obp.tile([128, GRP * 12], F32, tag="ob",
                                   name=f"obA{g}")
                    obB = obp.tile([128, GRP * 12], F32, tag="ob",
                                   name=f"obB{g}")
                    nc.scalar.activation(obA[:], mA[:], AFT.Copy)
                    nc.scalar.activation(obB[:], mB[:], AFT.Copy)
                    nc.sync.dma_start(out[:, g * 96:g * 96 + 48], obA[:])
                    nc.sync.dma_start(out[:, g * 96 + 48:g * 96 + 96], obB[:])

            for s in range(NSUP + GRP):
                if s < NSUP:
                    emit_front(s)
                if s >= GRP:
                    emit_back(s - GRP)

    nc.compile()
    return nc


def _host_prep(x, mu, chol, alpha, rgb, rotation, translation, projection,
               bg):
    d32 = (mu.astype(np.float32) - translation.astype(np.float32)[None, :])
    dist = np.sqrt(np.sum(d32 * d32, axis=-1, dtype=np.float32))
    order = np.argsort(dist, kind="stable")
    mu = mu.astype(np.float64)[order]
    chol = chol.astype(np.float64)[order]
    alpha = alpha.astype(np.float64)[order]
    rgb = rgb.astype(np.float64)[order]
    rotation = rotation.astype(np.float64)
    translation = translation.astype(np.float64)
    projection = projection.astype(np.float64)
    bg = bg.astype(np.float64)

    inv_rot = rotation.T
    inv_trans = -inv_rot @ translation
    Lg = np.tril(chol) + 0.3 * np.eye(3)
    Sigma = np.einsum("gij,gkj->gik", Lg, Lg)
    mu_cam = np.einsum("ij,gj->gi", inv_rot, mu) + inv_trans
    mu2d = np.einsum("ij,gj->gi", projection, mu_cam)
    P_cam = projection @ inv_rot
    S2 = np.einsum("ij,gjk,lk->gil", P_cam, Sigma, P_cam) + 1e-4 * np.eye(2)
    det = S2[:, 0, 0] * S2[:, 1, 1] - S2[:, 0, 1] * S2[:, 1, 0]
    inv = np.empty((G, 2, 2))
    inv[:, 0, 0] = S2[:, 1, 1]
    inv[:, 0, 1] = -S2[:, 0, 1]
    inv[:, 1, 0] = -S2[:, 1, 0]
    inv[:, 1, 1] = S2[:, 0, 0]
    inv /= det[:, None, None]

    sp_ = np.logaddexp(0.0, alpha)
    wg = sp_ / (1.0 + sp_)
    color = rgb / (1.0 + np.abs(rgb))

    A = inv[:, 0, 0]
    Bc = inv[:, 0, 1] + inv[:, 1, 0]
    C = inv[:, 1, 1]
    m0, m1 = mu2d[:, 0], mu2d[:, 1]
    D = -2 * A * m0 - Bc * m1
    E = -Bc * m0 - 2 * C * m1
    F = A * m0 ** 2 + Bc * m0 * m1 + C * m1 ** 2
    coeffs = -0.5 * np.stack([A, Bc, C, D, E, F], axis=1)
    coeffs[:, 5] += np.log(wg)

    coeffsR = coeffs[::-1].copy()
    colorR = color[::-1].copy()
    mu2dR = mu2d[::-1]
    wgR = wg[::-1]
    trcR = (A + C)[::-1]
    dt2R = (A * C - (Bc / 2) ** 2)[::-1]
    lminR = (trcR - np.sqrt(np.maximum(trcR * trcR - 4 * dt2R, 0))) / 2

    xf = x.reshape(BN, 2).astype(np.float64)
    NB_ = 16
    bx = np.clip(((xf[:, 0] + 2) / 4 * NB_).astype(int), 0, NB_ - 1)
    by = np.clip(((xf[:, 1] + 2) / 4 * NB_).astype(int), 0, NB_ - 1)
    pix_order = np.argsort(by * NB_ + bx, kind="stable")
    xs = xf[pix_order]

    tiles = xs.reshape(NTG, 128, 2)
    tmin = tiles.min(axis=1)
    tmax = tiles.max(axis=1)
    cx = np.clip(mu2dR[None, :, 0], tmin[:, None, 0], tmax[:, None, 0])
    cy = np.clip(mu2dR[None, :, 1], tmin[:, None, 1], tmax[:, None, 1])
    dd = (mu2dR[None, :, 0] - cx) ** 2 + (mu2dR[None, :, 1] - cy) ** 2
    abound = wgR[None, :] * np.exp(-0.5 * lminR[None, :] * dd)
    sel = np.argsort(-abound, axis=1, kind="stable")[:, :GA]
    sel.sort(axis=1)

    coefT = np.ascontiguousarray(coeffsR.T).astype(np.float32)
    ch = coefT.astype(BF)
    cl = (coefT - ch.astype(np.float32)).astype(BF)
    c18 = np.concatenate([ch, ch, cl], axis=0).astype(np.float32)
    c18a = c18[:, sel.reshape(-1)].astype(BF)          # [18, NTG*GA]

    csel = colorR[sel]                                 # [NTG, GA, 3]
    dtile = np.empty((NTG, GA, 3))
    dtile[:, :-1] = csel[:, 1:] - csel[:, :-1]
    dtile[:, -1] = bg[None, :] - csel[:, -1]
    dtile[:, 1::2] *= -1.0
    offs = csel[:, 0].astype(np.float32)

    zconst = np.zeros((128, WGRP), NPF16)
    for i in range(GRP * SS):
        zconst[:, i * BLK + GA] = -1.0

    return (c18a, dtile.astype(NPF16), zconst, np.eye(128, dtype=NPF16),
            offs, pix_order, xs)


def kernel(x, mu, chol, alpha, rgb, rotation, translation, projection,
           background_color):
    global _cached, LAST_EXEC_NS, LAST_RESULTS
    x = np.asarray(x, np.float32)
    c18a, dtile, zconst, ident, offs, pix_order, xs = _host_prep(
        x, np.asarray(mu), np.asarray(chol), np.asarray(alpha),
        np.asarray(rgb), np.asarray(rotation), np.asarray(translation),
        np.asarray(projection), np.asarray(background_color))

    feat = np.empty((6, BN), np.float32)
    feat[0] = xs[:, 0] ** 2
    feat[1] = xs[:, 0] * xs[:, 1]
    feat[2] = xs[:, 1] ** 2
    feat[3] = xs[:, 0]
    feat[4] = xs[:, 1]
    feat[5] = 1.0
    fh = feat.astype(BF)
    fl = (feat - fh.astype(np.float32)).astype(BF)
    f18 = np.concatenate([fh, fl, fh], axis=0)

    if _cached is None:
        _cached = _build()
    nc = _cached

    in_maps = []
    for k in range(NCORES):
        d16a = np.zeros((128, NT * 3), NPF16)
        dk = dtile[k * NT:(k + 1) * NT]                # [NT, GA, 3]
        dk2 = dk.transpose(1, 0, 2).reshape(GA, NT * 3)
        d16a[0:GA] = dk2
        d16a[64:64 + GA] = dk2
        in_maps.append({
            "f18": np.ascontiguousarray(f18[:, k * PPC:(k + 1) * PPC]),
            "c18a": np.ascontiguousarray(
                c18a[:, k * NT * GA:(k + 1) * NT * GA]),
            "d16a": d16a,
            "zconst": zconst,
            "ident": ident,
        })

    kwargs = {}
    if PROFILE:
        kwargs = dict(trace=True)
    res = run_bass_kernel_spmd(nc, in_maps, core_ids=list(range(NCORES)),
                               **kwargs)
    LAST_EXEC_NS = res.exec_time_ns
    LAST_RESULTS = res

    parts = []
    for k in range(NCORES):
        # out cols: [group, parity, j, pair, c]; px tile = group*32 +
        # j*8 + pair*2 + parity
        arr = res.results[k]["out"].reshape(128, NGRP, 2, GRP, SS // 2, 3)
        arr = arr.transpose(5, 1, 3, 4, 2, 0)  # [3, g, j, pair, parity, lane]
        parts.append(arr.reshape(3, PPC))
    sortedv = np.concatenate(parts, axis=1)
    sortedv = sortedv + offs.T.repeat(128, axis=1)
    outp = np.empty_like(sortedv)
    outp[:, pix_order] = sortedv
    return outp.T.reshape(B, N, 3).astype(np.float32)


# revision 5
# speedup vs baseline: 1.1357x; 1.0835x over previous
"""Gaussian-splat blend kernel v4 — scan + culling + paired transposes.

Like v3 (per-tile top-GA gaussian culling, scan-based transmittance) with:
  - GA=63, blocks of 64 (63 real + 1 reset pad): two px-tiles' T-blocks
    fit one [128,128] PE transpose -> half the transposes, half the
    PSUM->SBUF copy volume.
  - Even px-tiles contract st rows 0:63, odd rows 64:127; their mm3
    outputs go to SEPARATE PSUM banks (PE row-strip mixing within one
    bank faults the device).
  - exp merged over 2 supersteps ([128, 1008] PSUM reads).
"""

import numpy as np
import ml_dtypes

import concourse.bass as bass
import concourse.bacc as bacc
import concourse.mybir as mybir
import concourse.tile as tile
from concourse.bass_utils import run_bass_kernel_spmd

G = 128
GA = 63                     # gaussians kept per tile
B = 4
N = 65536
BN = B * N
NCORES = 8
PPC = BN // NCORES          # 32768
NT = PPC // 128             # 256 tiles/core
NTG = BN // 128             # 2048 tiles global
SS = 8                      # px tiles per superstep
NSUP = NT // SS             # 32
SUPPX = SS * 128            # 1024
BLK = GA + 1                # 64
GRP = 4
NGRP = NSUP // GRP          # 8
WSS = SS * BLK              # 512
WGRP = GRP * WSS            # 2048
CSPLIT = 512                # stp copy cols on ACT (rest on DVE)

F32 = mybir.dt.float32
BF16 = mybir.dt.bfloat16
F16 = mybir.dt.float16
AFT = mybir.ActivationFunctionType
ALU = mybir.AluOpType
BF = ml_dtypes.bfloat16
NPF16 = np.float16

PROFILE = False
LAST_EXEC_NS = None
LAST_RESULTS = None

_cached = None


def _build():
    nc = bacc.Bacc("TRN2", target_bir_lowering=False, debug=False,
                   num_devices=NCORES)
    f18 = nc.dram_tensor("f18", [18, PPC], BF16, kind="ExternalInput")
    c18a = nc.dram_tensor("c18a", [18, NT * GA], BF16, kind="ExternalInput")
    d16a = nc.dram_tensor("d16a", [128, NT * 3], F16, kind="ExternalInput")
    zconst = nc.dram_tensor("zconst", [128, WGRP], F16, kind="ExternalInput")
    ident = nc.dram_tensor("ident", [128, 128], F16, kind="ExternalInput")
    # per group: [128, 48] even-parity block then [128, 48] odd
    out = nc.dram_tensor("out", [128, NGRP * 96], F32, kind="ExternalOutput")

    with tile.TileContext(nc) as tc:
        with (
            tc.tile_pool(name="const", bufs=1) as constp,
            tc.tile_pool(name="featp", bufs=4) as featp,
            tc.tile_pool(name="cbp", bufs=4) as cbp,
            tc.tile_pool(name="zp", bufs=3, space="PSUM") as zp,
            tc.tile_pool(name="stpp", bufs=3, space="PSUM") as stpp,
            tc.tile_pool(name="moutp", bufs=1, space="PSUM") as moutp,
            tc.tile_pool(name="btp", bufs=2) as btp,
            tc.tile_pool(name="tp", bufs=2) as tp_,
            tc.tile_pool(name="stp", bufs=3) as stp_,
            tc.tile_pool(name="obp", bufs=2) as obp,
        ):
            dummy = constp.tile([1, 8], F32)
            nc.gpsimd.memset(dummy[:], 0.0)
            nc.scalar.activation(dummy[:], dummy[:], AFT.Exp)

            fbufs = [featp.tile([18, SUPPX], BF16, tag="fbuf",
                                name=f"fbuf{i}") for i in range(NSUP)]
            cbufs = [cbp.tile([18, SS * GA], BF16, tag="cbuf",
                              name=f"cbuf{i}") for i in range(NSUP)]
            nc.sync.dma_start(fbufs[0][:], f18[:, bass.ts(0, SUPPX)])
            nc.sync.dma_start(cbufs[0][:], c18a[:, bass.ts(0, SS * GA)])
            d16_t = constp.tile([128, NT * 3], F16)
            nc.sync.dma_start(d16_t[:], d16a[:])
            zc_t = constp.tile([128, WGRP], F16)
            nc.gpsimd.dma_start(zc_t[:], zconst[:])
            id_t = constp.tile([128, 128], F16)
            nc.gpsimd.dma_start(id_t[:], ident[:])

            a4s = [constp.tile([128, GRP * SS, BLK], F16, name=f"a4_{i}")
                   for i in range(2)]
            for a4 in a4s:
                nc.gpsimd.memset(a4[:, :, GA:BLK], 1.0)

            tts = {}
            state = {}

            def emit_front(s):
                fbuf, cbuf = fbufs[s], cbufs[s]
                if s > 0:
                    nc.gpsimd.dma_start(fbuf[:], f18[:, bass.ts(s, SUPPX)])
                    nc.sync.dma_start(cbuf[:], c18a[:, bass.ts(s, SS * GA)])
                z2 = zp.tile([128, SS * GA], F32, name="z2")
                for i in range(SS):
                    nc.tensor.matmul(
                        z2[:, bass.ts(i, GA)],
                        fbuf[:, bass.ts(i, 128)],
                        cbuf[:, bass.ts(i, GA)], start=True, stop=True)
                a4 = a4s[(s // GRP) % 2]
                b0 = (s % GRP) * SS
                nc.scalar.activation(
                    a4[:, b0:b0 + SS, 0:GA],
                    z2.rearrange("p (b c) -> p b c", c=GA)[:], AFT.Exp)
                # per-superstep b-pass: the group scan only waits on the
                # last quarter instead of a full-group subtract
                if s % GRP == 0:
                    state['bt'] = btp.tile([128, WGRP], F16, name="bt")
                bt = state['bt']
                a4f = a4.rearrange("p b c -> p (b c)")
                qw = WGRP // GRP
                q0 = (s % GRP) * qw
                nc.vector.tensor_scalar_sub(
                    bt[:, q0:q0 + qw], a4f[:, q0:q0 + qw], 1.0)
                # blocks reset the scan state at their pad column, so the
                # group scan splits at any block boundary with initial=-1:
                # run half-group scans so transposes start half a group
                # earlier
                if s % GRP == 1:
                    tt = tp_.tile([128, WGRP], F16)
                    tts[s // GRP] = tt
                    nc.vector.tensor_tensor_scan(
                        tt[:, 0:WGRP // 2], bt[:, 0:WGRP // 2],
                        zc_t[:, 0:WGRP // 2],
                        initial=-1.0, op0=ALU.mult, op1=ALU.add)
                elif s % GRP == GRP - 1:
                    tt = tts[s // GRP]
                    nc.vector.tensor_tensor_scan(
                        tt[:, WGRP // 2:WGRP], bt[:, WGRP // 2:WGRP],
                        zc_t[:, WGRP // 2:WGRP],
                        initial=-1.0, op0=ALU.mult, op1=ALU.add)

            def emit_back(s):
                tt = tts[s // GRP]
                off = (s % GRP) * WSS
                stp = stpp.tile([128, SS // 2 * 128], F16, name="stp")
                for p in range(SS // 2):
                    nc.tensor.transpose(
                        stp[:, bass.ts(p, 128)],
                        tt[:, off + p * 128:off + (p + 1) * 128], id_t[:])
                st = stp_.tile([128, SS // 2 * 128], F16, name="st")
                nc.scalar.activation(st[:, 0:CSPLIT], stp[:, 0:CSPLIT],
                                     AFT.Copy)
                if CSPLIT < SS // 2 * 128:
                    nc.vector.tensor_copy(st[:, CSPLIT:], stp[:, CSPLIT:])
                j = s % GRP
                if j == 0:
                    state['mA'] = moutp.tile([128, GRP * 12], F32,
                                             name="moutA")
                    state['mB'] = moutp.tile([128, GRP * 12], F32,
                                             name="moutB")
                mA, mB = state['mA'], state['mB']
                for p in range(SS // 2):
                    t_even = s * SS + 2 * p
                    t_odd = t_even + 1
                    nc.tensor.matmul(
                        mA[:, j * 12 + p * 3:j * 12 + p * 3 + 3],
                        st[0:GA, bass.ts(p, 128)],
                        d16_t[0:GA, t_even * 3:t_even * 3 + 3],
                        start=True, stop=True)
                    nc.tensor.matmul(
                        mB[:, j * 12 + p * 3:j * 12 + p * 3 + 3],
                        st[64:64 + GA, bass.ts(p, 128)],
                        d16_t[64:64 + GA, t_odd * 3:t_odd * 3 + 3],
                        start=True, stop=True)
                if j == GRP - 1:
                    g = s // GRP
                    obA = obp.tile([128, GRP * 12], F32, tag="ob",
                                   name=f"obA{g}")
                    obB = obp.tile([128, GRP * 12], F32, tag="ob",
                                   name=f"obB{g}")
                    nc.scalar.activation(obA[:], mA[:], AFT.Copy)
                    nc.scalar.activation(obB[:], mB[:], AFT.Copy)
                    nc.sync.dma_start(out[:, g * 96:g * 96 + 48], obA[:])
                    nc.sync.dma_start(out[:, g * 96 + 48:g * 96 + 96], obB[:])

            for s in range(NSUP + GRP):
                if s < NSUP:
                    emit_front(s)
                if s >= GRP:
                    emit_back(s - GRP)

    nc.compile()
    return nc


def _host_prep(x, mu, chol, alpha, rgb, rotation, translation, projection,
               bg):
    d32 = (mu.astype(np.float32) - translation.astype(np.float32)[None, :])
    dist = np.sqrt(np.sum(d32 * d32, axis=-1, dtype=np.float32))
    order = np.argsort(dist, kind="stable")
    mu = mu.astype(np.float64)[order]
    chol = chol.astype(np.float64)[order]
    alpha = alpha.astype(np.float64)[order]
    rgb = rgb.astype(np.float64)[order]
    rotation = rotation.astype(np.float64)
    translation = translation.astype(np.float64)
    projection = projection.astype(np.float64)
    bg = bg.astype(np.float64)

    inv_rot = rotation.T
    inv_trans = -inv_rot @ translation
    Lg = np.tril(chol) + 0.3 * np.eye(3)
    Sigma = np.einsum("gij,gkj->gik", Lg, Lg)
    mu_cam = np.einsum("ij,gj->gi", inv_rot, mu) + inv_trans
    mu2d = np.einsum("ij,gj->gi", projection, mu_cam)
    P_cam = projection @ inv_rot
    S2 = np.einsum("ij,gjk,lk->gil", P_cam, Sigma, P_cam) + 1e-4 * np.eye(2)
    det = S2[:, 0, 0] * S2[:, 1, 1] - S2[:, 0, 1] * S2[:, 1, 0]
    inv = np.empty((G, 2, 2))
    inv[:, 0, 0] = S2[:, 1, 1]
    inv[:, 0, 1] = -S2[:, 0, 1]
    inv[:, 1, 0] = -S2[:, 1, 0]
    inv[:, 1, 1] = S2[:, 0, 0]
    inv /= det[:, None, None]

    sp_ = np.logaddexp(0.0, alpha)
    wg = sp_ / (1.0 + sp_)
    color = rgb / (1.0 + np.abs(rgb))

    A = inv[:, 0, 0]
    Bc = inv[:, 0, 1] + inv[:, 1, 0]
    C = inv[:, 1, 1]
    m0, m1 = mu2d[:, 0], mu2d[:, 1]
    D = -2 * A * m0 - Bc * m1
    E = -Bc * m0 - 2 * C * m1
    F = A * m0 ** 2 + Bc * m0 * m1 + C * m1 ** 2
    coeffs = -0.5 * np.stack([A, Bc, C, D, E, F], axis=1)
    coeffs[:, 5] += np.log(wg)

    coeffsR = coeffs[::-1].copy()
    colorR = color[::-1].copy()
    mu2dR = mu2d[::-1]
    wgR = wg[::-1]
    trcR = (A + C)[::-1]
    dt2R = (A * C - (Bc / 2) ** 2)[::-1]
    lminR = (trcR - np.sqrt(np.maximum(trcR * trcR - 4 * dt2R, 0))) / 2

    xf = x.reshape(BN, 2).astype(np.float64)
    NB_ = 16
    bx = np.clip(((xf[:, 0] + 2) / 4 * NB_).astype(int), 0, NB_ - 1)
    by = np.clip(((xf[:, 1] + 2) / 4 * NB_).astype(int), 0, NB_ - 1)
    pix_order = np.argsort(by * NB_ + bx, kind="stable")
    xs = xf[pix_order]

    tiles = xs.reshape(NTG, 128, 2)
    tmin = tiles.min(axis=1)
    tmax = tiles.max(axis=1)
    cx = np.clip(mu2dR[None, :, 0], tmin[:, None, 0], tmax[:, None, 0])
    cy = np.clip(mu2dR[None, :, 1], tmin[:, None, 1], tmax[:, None, 1])
    dd = (mu2dR[None, :, 0] - cx) ** 2 + (mu2dR[None, :, 1] - cy) ** 2
    abound = wgR[None, :] * np.exp(-0.5 * lminR[None, :] * dd)
    sel = np.argsort(-abound, axis=1, kind="stable")[:, :GA]
    sel.sort(axis=1)

    coefT = np.ascontiguousarray(coeffsR.T).astype(np.float32)
    ch = coefT.astype(BF)
    cl = (coefT - ch.astype(np.float32)).astype(BF)
    c18 = np.concatenate([ch, ch, cl], axis=0).astype(np.float32)
    c18a = c18[:, sel.reshape(-1)].astype(BF)          # [18, NTG*GA]

    csel = colorR[sel]                                 # [NTG, GA, 3]
    dtile = np.empty((NTG, GA, 3))
    dtile[:, :-1] = csel[:, 1:] - csel[:, :-1]
    dtile[:, -1] = bg[None, :] - csel[:, -1]
    dtile[:, 1::2] *= -1.0
    offs = csel[:, 0].astype(np.float32)

    zconst = np.zeros((128, WGRP), NPF16)
    for i in range(GRP * SS):
        zconst[:, i * BLK + GA] = -1.0

    return (c18a, dtile.astype(NPF16), zconst, np.eye(128, dtype=NPF16),
            offs, pix_order, xs)


def kernel(x, mu, chol, alpha, rgb, rotation, translation, projection,
           background_color):
    global _cached, LAST_EXEC_NS, LAST_RESULTS
    x = np.asarray(x, np.float32)
    c18a, dtile, zconst, ident, offs, pix_order, xs = _host_prep(
        x, np.asarray(mu), np.asarray(chol), np.asarray(alpha),
        np.asarray(rgb), np.asarray(rotation), np.asarray(translation),
        np.asarray(projection), np.asarray(background_color))

    feat = np.empty((6, BN), np.float32)
    feat[0] = xs[:, 0] ** 2
    feat[1] = xs[:, 0] * xs[:, 1]
    feat[2] = xs[:, 1] ** 2
    feat[3] = xs[:, 0]
    feat[4] = xs[:, 1]
    feat[5] = 1.0
    fh = feat.astype(BF)
    fl = (feat - fh.astype(np.float32)).astype(BF)
    f18 = np.concatenate([fh, fl, fh], axis=0)

    if _cached is None:
        _cached = _build()
    nc = _cached

    in_maps = []
    for k in range(NCORES):
        d16a = np.zeros((128, NT * 3), NPF16)
        dk = dtile[k * NT:(k + 1) * NT]                # [NT, GA, 3]
        dk2 = dk.transpose(1, 0, 2).reshape(GA, NT * 3)
        d16a[0:GA] = dk2
        d16a[64:64 + GA] = dk2
        in_maps.append({
            "f18": np.ascontiguousarray(f18[:, k * PPC:(k + 1) * PPC]),
            "c18a": np.ascontiguousarray(
                c18a[:, k * NT * GA:(k + 1) * NT * GA]),
            "d16a": d16a,
            "zconst": zconst,
            "ident": ident,
        })

    kwargs = {}
    if PROFILE:
        kwargs = dict(trace=True)
    res = run_bass_kernel_spmd(nc, in_maps, core_ids=list(range(NCORES)),
                               **kwargs)
    LAST_EXEC_NS = res.exec_time_ns
    LAST_RESULTS = res

    parts = []
    for k in range(NCORES):
        # out cols: [group, parity, j, pair, c]; px tile = group*32 +
        # j*8 + pair*2 + parity
        arr = res.results[k]["out"].reshape(128, NGRP, 2, GRP, SS // 2, 3)
        arr = arr.transpose(5, 1, 3, 4, 2, 0)  # [3, g, j, pair, parity, lane]
        parts.append(arr.reshape(3, PPC))
    sortedv = np.concatenate(parts, axis=1)
    sortedv = sortedv + offs.T.repeat(128, axis=1)
    outp = np.empty_like(sortedv)
    outp[:, pix_order] = sortedv
    return outp.T.reshape(B, N, 3).astype(np.float32)
